# revision 17
# baseline (speedup 1.0000x reference)
"""Trainium2 Bass kernel for nn_Confidence_Score (gnn_message_passing).

Math: with S_g = sum of x over nodes of graph g and n_g = node count,
every node of graph g has identical activations:
    h1_g = relu(S_g @ W1 + b1)
    h2_g = relu((n_g * h1_g) @ W2 + b2)
    c_g  = h2_g @ Wc + bc ;  out_node = sp/(1+sp), sp = softplus(c_g)

The kernel is DMA-stream-bound (per-ring ~100 GB/s), so v2 minimizes
HBM bytes: x is sent bf16-only (tolerance 2e-2 >> bf16 error), the
one-hot transpose A_T is generated on-chip from per-graph node bounds
(iota + two compares) instead of a 1.8MB broadcast DMA, and all MLP
weights are bf16.  Aggregation matmuls use the x-chunk as PE weights
and stream the 72-wide one-hot, so S lands transposed for the MLP.
Output head is sigmoid(ln(softplus(c+bc))) on the Act engine.

Sharding: graph-aligned contiguous node ranges, balanced by node count,
one range per core (8 cores); weights replicated; no collectives.
"""

import os
import sys

for _p in ("/root/.axon_site", "/root/.axon_site/_ro/trn_rl_repo",
           "/root/.axon_site/_ro/pypackages", "/opt/trn_rl_repo"):
    if os.path.isdir(_p) and _p not in sys.path:
        sys.path.append(_p)

import numpy as np

N_CORES = 8
D = 128
H = 256
G_TOTAL = 512
G_PAD = 72        # max local graphs per core (actual ~66)
CHUNK = 128       # nodes per aggregation matmul
XB = 10           # chunks per x DMA group
ROW = D + 2       # x bf16 | [0, bt] pair (bitcast-readable as f32)
OB = 512          # nodes per expansion matmul
TB = 1280         # A_T generation block width
NB = 3            # output bands (32 partitions each; base 96 is illegal)

# wpk (bf16 consts) column layout
W_W1 = 0
W_W2A = 256
W_W2B = 512
W_WC = 768        # wc as 2 cols: rows 0:128 and 128:256
W_IO = 770        # iota72 [128, 72]: value g at col g
W_B1 = 842        # b1 broadcast [72, 256]
W_B2 = 1098
WPKC = 1354

# cpk (f32 consts) column layout: ncs | bcs | lo[n_tb] | hi[n_tb] | ident
C_NC = 0
C_BC = 1
C_LO = 2

_CACHE = {}


def _build(nodes_pad):
    """Build + compile the single-core Bass program (shapes uniform across cores)."""
    from contextlib import ExitStack

    import concourse.bacc as bacc
    import concourse.mybir as mybir
    import concourse.tile as tile

    f32 = mybir.dt.float32
    bf16 = mybir.dt.bfloat16
    u8 = mybir.dt.uint8
    AF = mybir.ActivationFunctionType
    OP = mybir.AluOpType

    n_chunks = nodes_pad // CHUNK
    n_groups = n_chunks // XB
    n_ob = nodes_pad // OB
    n_tb = nodes_pad // TB
    NQ = -(-n_ob // NB)
    cpkc = 3 + G_PAD

    nc = bacc.Bacc("TRN2", target_bir_lowering=False, debug=False)

    xb_d = nc.dram_tensor("xb", [nodes_pad, ROW], bf16, kind="ExternalInput").ap()
    bt8_d = nc.dram_tensor("bt8", [1, nodes_pad], u8, kind="ExternalInput").ap()
    wpk_d = nc.dram_tensor("wpk", [128, WPKC], bf16, kind="ExternalInput").ap()
    cpk_d = nc.dram_tensor("cpk", [128, cpkc], f32, kind="ExternalInput").ap()
    out_d = nc.dram_tensor("out", [n_ob, OB], f32, kind="ExternalOutput").ap()

    # host pre-shuffles xb so each (group, partition) segment is contiguous
    xb_groups = xb_d.rearrange("(g p j) d -> g p (j d)", p=CHUNK, j=XB)

    with tile.TileContext(nc) as tc, ExitStack() as ctx:
        const = ctx.enter_context(tc.tile_pool(name="const", bufs=1))
        store = ctx.enter_context(tc.tile_pool(name="store", bufs=1))
        ps_s = ctx.enter_context(tc.tile_pool(name="ps_s", bufs=1, space="PSUM"))

        wpk = const.tile([128, WPKC], bf16)
        nc.scalar.dma_start(wpk[:], wpk_d[:])
        cpk = const.tile([128, cpkc], f32)
        nc.gpsimd.dma_start(cpk[:], cpk_d[:])

        w1 = wpk[:, W_W1:W_W1 + H]
        w2a = wpk[:, W_W2A:W_W2A + H]
        w2b = wpk[:, W_W2B:W_W2B + H]
        wca = wpk[:, W_WC:W_WC + 1]
        wcb = wpk[:, W_WC + 1:W_WC + 2]
        io72 = wpk[:, W_IO:W_IO + G_PAD]
        b1s = wpk[0:G_PAD, W_B1:W_B1 + H]
        b2s = wpk[0:G_PAD, W_B2:W_B2 + H]
        ncs = cpk[0:G_PAD, C_NC:C_NC + 1]
        bcs = cpk[0:G_PAD, C_BC:C_BC + 1]
        ioc = cpk[0:G_PAD, 2:3]
        identf = cpk[0:G_PAD, 3:3 + G_PAD]

        at_sb = store.tile([G_PAD, nodes_pad], bf16)
        es2 = store.tile([96, NQ * OB], f32)
        btb8 = store.tile([G_PAD, nodes_pad], u8)
        s_ps = ps_s.tile([D, G_PAD], f32)
        ogr = const.tile([G_PAD, 32], bf16)

        # ---- all x DMAs issued up-front on 3 rings ----
        xpool = ctx.enter_context(tc.tile_pool(name="xp", bufs=n_groups))
        xts = []
        half = nodes_pad // 2
        for g in range(n_groups):
            xt = xpool.tile([CHUNK, XB * ROW], bf16)
            eng = (nc.sync, nc.scalar, nc.gpsimd)[g % 3] if g < 9 else nc.scalar
            eng.dma_start(xt[:], xb_groups[g])
            xts.append(xt)
            if g == 0:
                nc.sync.dma_start(
                    btb8[:, 0:half],
                    bt8_d[0:1, 0:half].to_broadcast((G_PAD, half)))
            if g == 2:
                nc.gpsimd.dma_start(
                    btb8[:, half:],
                    bt8_d[0:1, half:].to_broadcast((G_PAD, nodes_pad - half)))

        def at_gen(k, eng):
            """A_T[g, n] = (bt[n] == g) for block k (one is_eq op)."""
            eng.tensor_scalar(
                at_sb[:, k * TB:(k + 1) * TB],
                btb8[:, k * TB:(k + 1) * TB], ioc, None, op0=OP.is_equal,
            )

        # ---- pass 1: segment-sum via one-hot matmuls; A_T gen interleaved ----
        with tc.tile_pool(name="ap", bufs=4) as apool:
            for g in range(n_groups):
                xt = xts[g]
                a2 = apool.tile([CHUNK, XB * G_PAD], bf16)
                # one-hot for all XB chunks of the group in one Pool op:
                # a[p, j, g] = (iota72[p, g] == bt[p, j])
                bts = (xt[:].rearrange("p (j r) -> p j r", r=ROW)
                       [:, :, D + 1:D + 2])
                nc.vector.tensor_tensor(
                    a2[:].rearrange("p (j g) -> p j g", g=G_PAD),
                    io72.rearrange("p (o g) -> p o g", o=1)
                        .to_broadcast((CHUNK, XB, G_PAD)),
                    bts.to_broadcast((CHUNK, XB, G_PAD)),
                    op=OP.is_equal,
                )
                for j in range(XB):
                    c = g * XB + j
                    nc.tensor.matmul(
                        s_ps[:],
                        lhsT=xt[:, j * ROW:j * ROW + D],
                        rhs=a2[:, j * G_PAD:(j + 1) * G_PAD],
                        start=(c == 0), stop=(c == n_chunks - 1),
                    )
                # DVE: A_T blocks squeezed between matmul groups
                if g < n_tb:
                    at_gen(g, nc.vector)

        # ---- per-graph MLP (bf16 weights/activations, f32 accum) ----
        with (
            tc.tile_pool(name="mlp", bufs=1) as mlp,
            tc.tile_pool(name="ps_m", bufs=2, space="PSUM") as ps_m,
        ):
            st_bf = mlp.tile([D, G_PAD], bf16)
            nc.vector.tensor_copy(st_bf[:], s_ps[:])

            h1_ps = ps_m.tile([G_PAD, H], f32, tag="mm")
            nc.tensor.matmul(h1_ps[:], lhsT=st_bf[:], rhs=w1, start=True, stop=True)
            h1f = mlp.tile([G_PAD, H], f32)
            nc.vector.tensor_tensor(h1f[:], h1_ps[:], b1s, op=OP.add)
            h1b = mlp.tile([G_PAD, H], f32)
            nc.vector.tensor_scalar(h1b[:], h1f[:], 0.0, ncs, op0=OP.max, op1=OP.mult)

            tp1 = ps_m.tile([D, 2 * G_PAD], f32, tag="tp")
            for kk in range(2):
                nc.tensor.transpose(
                    tp1[:, kk * G_PAD:(kk + 1) * G_PAD],
                    h1b[:, kk * D:(kk + 1) * D], identf,
                )
            h1t = mlp.tile([D, 2 * G_PAD], bf16)
            nc.vector.tensor_copy(h1t[:], tp1[:])

            h2_ps = ps_m.tile([G_PAD, H], f32, tag="mm")
            nc.tensor.matmul(h2_ps[:], lhsT=h1t[:, 0:G_PAD], rhs=w2a,
                             start=True, stop=False)
            nc.tensor.matmul(h2_ps[:], lhsT=h1t[:, G_PAD:2 * G_PAD], rhs=w2b,
                             start=False, stop=True)
            h2f = mlp.tile([G_PAD, H], f32)
            nc.vector.tensor_tensor(h2f[:], h2_ps[:], b2s, op=OP.add)
            h2b = mlp.tile([G_PAD, H], f32)
            nc.vector.tensor_scalar_max(h2b[:], h2f[:], 0.0)

            tp2 = ps_m.tile([D, 2 * G_PAD], f32, tag="tp")
            for kk in range(2):
                nc.tensor.transpose(
                    tp2[:, kk * G_PAD:(kk + 1) * G_PAD],
                    h2b[:, kk * D:(kk + 1) * D], identf,
                )
            h2t = mlp.tile([D, 2 * G_PAD], bf16)
            nc.vector.tensor_copy(h2t[:], tp2[:])

            c_ps = ps_m.tile([G_PAD, 1], f32, tag="c")
            nc.tensor.matmul(c_ps[:], lhsT=h2t[:, 0:G_PAD], rhs=wca,
                             start=True, stop=False)
            nc.tensor.matmul(c_ps[:], lhsT=h2t[:, G_PAD:2 * G_PAD], rhs=wcb,
                             start=False, stop=True)

            # sp = softplus(c+bc) = relu(cc) + ln(1 + exp(-|cc|)); out = 1 - 1/(1+sp)
            cc = mlp.tile([G_PAD, 1], f32)
            nc.vector.tensor_scalar_add(cc[:], c_ps[:], bcs)
            ab = mlp.tile([G_PAD, 1], f32)
            nc.scalar.activation(ab[:], cc[:], AF.Abs)
            ex = mlp.tile([G_PAD, 1], f32)
            nc.scalar.activation(ex[:], ab[:], AF.Exp, scale=-1.0)
            lg = mlp.tile([G_PAD, 1], f32)
            nc.scalar.activation(lg[:], ex[:], AF.Ln, bias=1.0)
            rl = mlp.tile([G_PAD, 1], f32)
            nc.vector.tensor_scalar_max(rl[:], cc[:], 0.0)
            sp = mlp.tile([G_PAD, 1], f32)
            nc.vector.tensor_tensor(sp[:], rl[:], lg[:], op=OP.add)
            t1 = mlp.tile([G_PAD, 1], f32)
            nc.vector.tensor_scalar_add(t1[:], sp[:], 1.0)
            rcp = mlp.tile([G_PAD, 1], f32)
            nc.vector.reciprocal(rcp[:], t1[:])
            og = mlp.tile([G_PAD, 1], f32)
            nc.vector.tensor_scalar(og[:], rcp[:], -1.0, 1.0, op0=OP.mult, op1=OP.add)
            zz = mlp.tile([G_PAD, 32], f32)
            nc.vector.memset(zz[:], 0.0)
            nc.vector.tensor_scalar(ogr[:], zz[:], og[:], None, op0=OP.add)

        # ---- pass 2: out = og.T @ A_T; band r holds blocks r*NQ+q ----
        with tc.tile_pool(name="ps_e", bufs=3, space="PSUM") as ps_e:
            for q in range(NQ):
                e_ps = ps_e.tile([96, OB], f32)
                for r in range(NB):
                    b = r * NQ + q
                    if b >= n_ob:
                        continue
                    nc.tensor.matmul(
                        e_ps[32 * r:32 * r + 32, :], lhsT=ogr[:],
                        rhs=at_sb[:, b * OB:(b + 1) * OB],
                        start=True, stop=True,
                    )
                dst = es2[:, q * OB:(q + 1) * OB]
                if q % 2 == 0:
                    nc.vector.tensor_copy(dst, e_ps[:])
                else:
                    nc.scalar.copy(dst, e_ps[:])
            for r in range(NB):
                nb = min(NQ, n_ob - r * NQ)
                if nb <= 0:
                    continue
                eng = nc.sync if r % 2 == 0 else nc.scalar
                eng.dma_start(
                    out_d[r * NQ:r * NQ + nb, :].rearrange("a i -> (a i)"),
                    es2[32 * r:32 * r + 1, 0:nb * OB],
                )

    nc.compile()
    return nc


def _shard(batch):
    """Graph-aligned split of nodes across cores, balanced by node count."""
    n = batch.shape[0]
    counts = np.bincount(batch, minlength=G_TOTAL).astype(np.int64)
    bounds = np.concatenate([[0], np.cumsum(counts)])
    gsplit = [0]
    for k in range(1, N_CORES):
        t = k * n // N_CORES
        g = int(np.searchsorted(bounds, t))
        if g > 0 and abs(int(bounds[g - 1]) - t) < abs(int(bounds[g]) - t):
            g -= 1
        g = min(max(g, gsplit[-1]), G_TOTAL)
        gsplit.append(g)
    gsplit.append(G_TOTAL)
    return counts, bounds, gsplit


def kernel(**inputs):
    import ml_dtypes
    from concourse.bass_utils import run_bass_kernel_spmd

    bf16 = ml_dtypes.bfloat16
    x = np.ascontiguousarray(np.asarray(inputs["x"], dtype=np.float32))
    batch = np.asarray(inputs["batch"]).astype(np.int64)
    W1 = np.asarray(inputs["W1"], dtype=np.float32)
    b1 = np.asarray(inputs["b1"], dtype=np.float32)
    W2 = np.asarray(inputs["W2"], dtype=np.float32)
    b2 = np.asarray(inputs["b2"], dtype=np.float32)
    Wc = np.asarray(inputs["Wc"], dtype=np.float32).reshape(H, 1)
    bc = np.asarray(inputs["bc"], dtype=np.float32).reshape(1)

    n = batch.shape[0]
    counts, bounds, gsplit = _shard(batch)
    node_cnt = [int(bounds[gsplit[k + 1]] - bounds[gsplit[k]]) for k in range(N_CORES)]
    pad_unit = int(np.lcm.reduce([CHUNK * XB, OB, TB]))
    nodes_pad = int(-(-max(node_cnt) // pad_unit) * pad_unit)
    n_tb = nodes_pad // TB
    cpkc = 3 + G_PAD
    assert max(gsplit[k + 1] - gsplit[k] for k in range(N_CORES)) <= G_PAD

    key = nodes_pad
    if key not in _CACHE:
        _CACHE[key] = _build(nodes_pad)
    nc = _CACHE[key]

    wpk = np.zeros((128, WPKC), dtype=bf16)
    wpk[:, W_W1:W_W1 + H] = W1.astype(bf16)
    wpk[:, W_W2A:W_W2A + H] = W2[0:128].astype(bf16)
    wpk[:, W_W2B:W_W2B + H] = W2[128:256].astype(bf16)
    wpk[:, W_WC] = Wc[0:128, 0].astype(bf16)
    wpk[:, W_WC + 1] = Wc[128:256, 0].astype(bf16)
    wpk[:, W_IO:W_IO + G_PAD] = np.broadcast_to(
        np.arange(G_PAD, dtype=np.float32), (128, G_PAD)).astype(bf16)
    wpk[0:G_PAD, W_B1:W_B1 + H] = b1.astype(bf16)
    wpk[0:G_PAD, W_B2:W_B2 + H] = b2.astype(bf16)

    n_groups = nodes_pad // (CHUNK * XB)
    in_maps = []
    for k in range(N_CORES):
        gs, ge = gsplit[k], gsplit[k + 1]
        ns, ne = int(bounds[gs]), int(bounds[ge])
        cnt = ne - ns
        ng = ge - gs
        bt = np.full(nodes_pad, G_PAD - 1, dtype=np.float32)
        bt[:cnt] = (batch[ns:ne] - gs).astype(np.float32)
        xbp = np.zeros((nodes_pad, ROW), dtype=bf16)
        xbp[:cnt, :D] = x[ns:ne].astype(bf16)
        xbp[:, D + 1] = bt.astype(bf16)  # high half of an f32 via bitcast
        # shuffle to (group, partition, chunk-in-group, row) DMA order
        xbp = np.ascontiguousarray(
            xbp.reshape(n_groups, XB, CHUNK, ROW).transpose(0, 2, 1, 3)
        ).reshape(nodes_pad, ROW)

        cpk = np.zeros((128, cpkc), dtype=np.float32)
        cpk[0:ng, C_NC] = counts[gs:ge].astype(np.float32)
        cpk[:, C_BC] = bc[0]
        cpk[0:G_PAD, 2] = np.arange(G_PAD, dtype=np.float32)
        cpk[0:G_PAD, 3:3 + G_PAD] = np.eye(G_PAD, dtype=np.float32)
        bt8 = np.full((1, nodes_pad), 255, dtype=np.uint8)
        bt8[0, :cnt] = (batch[ns:ne] - gs).astype(np.uint8)
        in_maps.append({"xb": xbp, "bt8": bt8, "wpk": wpk, "cpk": cpk})

    res = run_bass_kernel_spmd(nc, in_maps, core_ids=list(range(N_CORES)))
    outs = []
    for k in range(N_CORES):
        o = res.results[k]["out"].reshape(-1)
        outs.append(o[: node_cnt[k]])
    return np.concatenate(outs).reshape(n, 1).astype(np.float32)


# revision 18
# speedup vs baseline: 1.1450x; 1.1450x over previous
"""Trainium2 Bass kernel for nn_Confidence_Score (gnn_message_passing).

Math: with S_g = sum of x over nodes of graph g and n_g = node count,
every node of graph g has identical activations:
    h1_g = relu(S_g @ W1 + b1)
    h2_g = relu((n_g * h1_g) @ W2 + b2)
    c_g  = h2_g @ Wc + bc ;  out_node = sp/(1+sp), sp = softplus(c_g)

The kernel is DMA-stream-bound (per-ring ~100 GB/s), so v2 minimizes
HBM bytes: x is sent bf16-only (tolerance 2e-2 >> bf16 error), the
one-hot transpose A_T is generated on-chip from per-graph node bounds
(iota + two compares) instead of a 1.8MB broadcast DMA, and all MLP
weights are bf16.  Aggregation matmuls use the x-chunk as PE weights
and stream the 72-wide one-hot, so S lands transposed for the MLP.
Output head is sigmoid(ln(softplus(c+bc))) on the Act engine.

Sharding: graph-aligned contiguous node ranges, balanced by node count,
one range per core (8 cores); weights replicated; no collectives.
"""

import os
import sys

for _p in ("/root/.axon_site", "/root/.axon_site/_ro/trn_rl_repo",
           "/root/.axon_site/_ro/pypackages", "/opt/trn_rl_repo"):
    if os.path.isdir(_p) and _p not in sys.path:
        sys.path.append(_p)

import numpy as np

N_CORES = 8
D = 128
H = 256
G_TOTAL = 512
G_PAD = 72        # max local graphs per core (actual ~66)
CHUNK = 128       # nodes per aggregation matmul
XB = 10           # chunks per x DMA group
ROW = D + 2       # x bf16 | [0, bt] pair (bitcast-readable as f32)
OB = 512          # nodes per expansion matmul
TB = 1280         # A_T generation block width
NB = 3            # output bands (32 partitions each; base 96 is illegal)

# wpk (bf16 consts) column layout
W_W1 = 0
W_W2A = 256
W_W2B = 512
W_WC = 768        # wc as 2 cols: rows 0:128 and 128:256
W_IO = 770        # iota72 [128, 72]: value g at col g
W_B1 = 842        # b1 broadcast [72, 256]
W_B2 = 1098
WPKC = 1354

# cpk (f32 consts) column layout: ncs | bcs | lo[n_tb] | hi[n_tb] | ident
C_NC = 0
C_BC = 1
C_LO = 2

_CACHE = {}


def _build(nodes_pad):
    """Build + compile the single-core Bass program (shapes uniform across cores)."""
    from contextlib import ExitStack

    import concourse.bacc as bacc
    import concourse.mybir as mybir
    import concourse.tile as tile

    f32 = mybir.dt.float32
    bf16 = mybir.dt.bfloat16
    u8 = mybir.dt.uint8
    AF = mybir.ActivationFunctionType
    OP = mybir.AluOpType

    n_chunks = nodes_pad // CHUNK
    n_groups = n_chunks // XB
    n_ob = nodes_pad // OB
    n_tb = nodes_pad // TB
    NQ = -(-n_ob // NB)
    cpkc = 3 + G_PAD

    nc = bacc.Bacc("TRN2", target_bir_lowering=False, debug=False)

    xb_d = nc.dram_tensor("xb", [nodes_pad, ROW], bf16, kind="ExternalInput").ap()
    bt8_d = nc.dram_tensor("bt8", [1, nodes_pad], u8, kind="ExternalInput").ap()
    iop_d = nc.dram_tensor("iopk", [128, G_PAD], bf16, kind="ExternalInput").ap()
    wpk_d = nc.dram_tensor("wpk", [128, WPKC], bf16, kind="ExternalInput").ap()
    cpk_d = nc.dram_tensor("cpk", [128, cpkc], f32, kind="ExternalInput").ap()
    out_d = nc.dram_tensor("out", [n_ob, OB], f32, kind="ExternalOutput").ap()

    # host pre-shuffles xb so each (group, partition) segment is contiguous
    xb_groups = xb_d.rearrange("(g p j) d -> g p (j d)", p=CHUNK, j=XB)

    with tile.TileContext(nc) as tc, ExitStack() as ctx:
        const = ctx.enter_context(tc.tile_pool(name="const", bufs=1))
        store = ctx.enter_context(tc.tile_pool(name="store", bufs=1))
        ps_s = ctx.enter_context(tc.tile_pool(name="ps_s", bufs=1, space="PSUM"))

        wpk = const.tile([128, WPKC], bf16)
        iop = const.tile([128, G_PAD], bf16)
        nc.scalar.dma_start(iop[:], iop_d[:])
        cpk = const.tile([128, cpkc], f32)
        nc.gpsimd.dma_start(cpk[:], cpk_d[:])

        w1 = wpk[:, W_W1:W_W1 + H]
        w2a = wpk[:, W_W2A:W_W2A + H]
        w2b = wpk[:, W_W2B:W_W2B + H]
        wca = wpk[:, W_WC:W_WC + 1]
        wcb = wpk[:, W_WC + 1:W_WC + 2]
        io72 = iop[:]
        b1s = wpk[0:G_PAD, W_B1:W_B1 + H]
        b2s = wpk[0:G_PAD, W_B2:W_B2 + H]
        ncs = cpk[0:G_PAD, C_NC:C_NC + 1]
        bcs = cpk[0:G_PAD, C_BC:C_BC + 1]
        ioc = cpk[0:G_PAD, 2:3]
        identf = cpk[0:G_PAD, 3:3 + G_PAD]

        at_sb = store.tile([G_PAD, nodes_pad], bf16)
        es2 = store.tile([96, NQ * OB], f32)
        btb8 = store.tile([G_PAD, nodes_pad], u8)
        s_ps = ps_s.tile([D, G_PAD], f32)
        ogr = const.tile([G_PAD, 32], bf16)

        # ---- all x DMAs issued up-front on 3 rings ----
        xpool = ctx.enter_context(tc.tile_pool(name="xp", bufs=n_groups))
        xts = []
        half = nodes_pad // 2
        for g in range(n_groups):
            xt = xpool.tile([CHUNK, XB * ROW], bf16)
            eng = (nc.sync, nc.scalar, nc.gpsimd)[g % 3] if g < 9 else nc.scalar
            eng.dma_start(xt[:], xb_groups[g])
            xts.append(xt)
            if g == 0:
                nc.sync.dma_start(
                    btb8[:, 0:half],
                    bt8_d[0:1, 0:half].to_broadcast((G_PAD, half)))
            if g == 2:
                nc.gpsimd.dma_start(
                    btb8[:, half:],
                    bt8_d[0:1, half:].to_broadcast((G_PAD, nodes_pad - half)))
            if g == n_groups - 1:
                nc.sync.dma_start(wpk[:], wpk_d[:])

        def at_gen(k, eng):
            """A_T[g, n] = (bt[n] == g) for block k (one is_eq op)."""
            eng.tensor_scalar(
                at_sb[:, k * TB:(k + 1) * TB],
                btb8[:, k * TB:(k + 1) * TB], ioc, None, op0=OP.is_equal,
            )

        # ---- pass 1: segment-sum via one-hot matmuls; A_T gen interleaved ----
        with tc.tile_pool(name="ap", bufs=4) as apool:
            for g in range(n_groups):
                xt = xts[g]
                a2 = apool.tile([CHUNK, XB * G_PAD], bf16)
                # one-hot for all XB chunks of the group in one Pool op:
                # a[p, j, g] = (iota72[p, g] == bt[p, j])
                bts = (xt[:].rearrange("p (j r) -> p j r", r=ROW)
                       [:, :, D + 1:D + 2])
                nc.vector.tensor_tensor(
                    a2[:].rearrange("p (j g) -> p j g", g=G_PAD),
                    io72.rearrange("p (o g) -> p o g", o=1)
                        .to_broadcast((CHUNK, XB, G_PAD)),
                    bts.to_broadcast((CHUNK, XB, G_PAD)),
                    op=OP.is_equal,
                )
                for j in range(XB):
                    c = g * XB + j
                    nc.tensor.matmul(
                        s_ps[:],
                        lhsT=xt[:, j * ROW:j * ROW + D],
                        rhs=a2[:, j * G_PAD:(j + 1) * G_PAD],
                        start=(c == 0), stop=(c == n_chunks - 1),
                    )
                # DVE: first half of A_T blocks squeezed between groups
                if g < n_tb // 2:
                    at_gen(g, nc.vector)
            for k in range(n_tb // 2, n_tb):
                at_gen(k, nc.vector)

        # ---- per-graph MLP (bf16 weights/activations, f32 accum) ----
        with (
            tc.tile_pool(name="mlp", bufs=1) as mlp,
            tc.tile_pool(name="ps_m", bufs=2, space="PSUM") as ps_m,
        ):
            st_bf = mlp.tile([D, G_PAD], bf16)
            nc.vector.tensor_copy(st_bf[:], s_ps[:])

            h1_ps = ps_m.tile([G_PAD, H], f32, tag="mm")
            nc.tensor.matmul(h1_ps[:], lhsT=st_bf[:], rhs=w1, start=True, stop=True)
            h1f = mlp.tile([G_PAD, H], f32)
            nc.vector.tensor_tensor(h1f[:], h1_ps[:], b1s, op=OP.add)
            h1b = mlp.tile([G_PAD, H], f32)
            nc.vector.tensor_scalar(h1b[:], h1f[:], 0.0, ncs, op0=OP.max, op1=OP.mult)

            tp1 = ps_m.tile([D, 2 * G_PAD], f32, tag="tp")
            for kk in range(2):
                nc.tensor.transpose(
                    tp1[:, kk * G_PAD:(kk + 1) * G_PAD],
                    h1b[:, kk * D:(kk + 1) * D], identf,
                )
            h1t = mlp.tile([D, 2 * G_PAD], bf16)
            nc.vector.tensor_copy(h1t[:], tp1[:])

            h2_ps = ps_m.tile([G_PAD, H], f32, tag="mm")
            nc.tensor.matmul(h2_ps[:], lhsT=h1t[:, 0:G_PAD], rhs=w2a,
                             start=True, stop=False)
            nc.tensor.matmul(h2_ps[:], lhsT=h1t[:, G_PAD:2 * G_PAD], rhs=w2b,
                             start=False, stop=True)
            h2f = mlp.tile([G_PAD, H], f32)
            nc.vector.tensor_tensor(h2f[:], h2_ps[:], b2s, op=OP.add)
            h2b = mlp.tile([G_PAD, H], f32)
            nc.vector.tensor_scalar_max(h2b[:], h2f[:], 0.0)

            tp2 = ps_m.tile([D, 2 * G_PAD], f32, tag="tp")
            for kk in range(2):
                nc.tensor.transpose(
                    tp2[:, kk * G_PAD:(kk + 1) * G_PAD],
                    h2b[:, kk * D:(kk + 1) * D], identf,
                )
            h2t = mlp.tile([D, 2 * G_PAD], bf16)
            nc.vector.tensor_copy(h2t[:], tp2[:])

            c_ps = ps_m.tile([G_PAD, 1], f32, tag="c")
            nc.tensor.matmul(c_ps[:], lhsT=h2t[:, 0:G_PAD], rhs=wca,
                             start=True, stop=False)
            nc.tensor.matmul(c_ps[:], lhsT=h2t[:, G_PAD:2 * G_PAD], rhs=wcb,
                             start=False, stop=True)

            # sp = softplus(c+bc) = relu(cc) + ln(1 + exp(-|cc|)); out = 1 - 1/(1+sp)
            cc = mlp.tile([G_PAD, 1], f32)
            nc.vector.tensor_scalar_add(cc[:], c_ps[:], bcs)
            ab = mlp.tile([G_PAD, 1], f32)
            nc.scalar.activation(ab[:], cc[:], AF.Abs)
            ex = mlp.tile([G_PAD, 1], f32)
            nc.scalar.activation(ex[:], ab[:], AF.Exp, scale=-1.0)
            lg = mlp.tile([G_PAD, 1], f32)
            nc.scalar.activation(lg[:], ex[:], AF.Ln, bias=1.0)
            rl = mlp.tile([G_PAD, 1], f32)
            nc.vector.tensor_scalar_max(rl[:], cc[:], 0.0)
            sp = mlp.tile([G_PAD, 1], f32)
            nc.vector.tensor_tensor(sp[:], rl[:], lg[:], op=OP.add)
            t1 = mlp.tile([G_PAD, 1], f32)
            nc.vector.tensor_scalar_add(t1[:], sp[:], 1.0)
            rcp = mlp.tile([G_PAD, 1], f32)
            nc.vector.reciprocal(rcp[:], t1[:])
            og = mlp.tile([G_PAD, 1], f32)
            nc.vector.tensor_scalar(og[:], rcp[:], -1.0, 1.0, op0=OP.mult, op1=OP.add)
            zz = mlp.tile([G_PAD, 32], f32)
            nc.vector.memset(zz[:], 0.0)
            nc.vector.tensor_scalar(ogr[:], zz[:], og[:], None, op0=OP.add)

        # ---- pass 2: out = og.T @ A_T; band r holds blocks r*NQ+q ----
        with tc.tile_pool(name="ps_e", bufs=3, space="PSUM") as ps_e:
            for q in range(NQ):
                e_ps = ps_e.tile([96, OB], f32)
                for r in range(NB):
                    b = r * NQ + q
                    if b >= n_ob:
                        continue
                    nc.tensor.matmul(
                        e_ps[32 * r:32 * r + 32, :], lhsT=ogr[:],
                        rhs=at_sb[:, b * OB:(b + 1) * OB],
                        start=True, stop=True,
                    )
                dst = es2[:, q * OB:(q + 1) * OB]
                if q % 2 == 0:
                    nc.vector.tensor_copy(dst, e_ps[:])
                else:
                    nc.scalar.copy(dst, e_ps[:])
            for r in range(NB):
                nb = min(NQ, n_ob - r * NQ)
                if nb <= 0:
                    continue
                eng = nc.sync if r % 2 == 0 else nc.scalar
                eng.dma_start(
                    out_d[r * NQ:r * NQ + nb, :].rearrange("a i -> (a i)"),
                    es2[32 * r:32 * r + 1, 0:nb * OB],
                )

    nc.compile()
    return nc


def _shard(batch):
    """Graph-aligned split of nodes across cores, balanced by node count."""
    n = batch.shape[0]
    counts = np.bincount(batch, minlength=G_TOTAL).astype(np.int64)
    bounds = np.concatenate([[0], np.cumsum(counts)])
    gsplit = [0]
    for k in range(1, N_CORES):
        t = k * n // N_CORES
        g = int(np.searchsorted(bounds, t))
        if g > 0 and abs(int(bounds[g - 1]) - t) < abs(int(bounds[g]) - t):
            g -= 1
        g = min(max(g, gsplit[-1]), G_TOTAL)
        gsplit.append(g)
    gsplit.append(G_TOTAL)
    return counts, bounds, gsplit


def kernel(**inputs):
    import ml_dtypes
    from concourse.bass_utils import run_bass_kernel_spmd

    bf16 = ml_dtypes.bfloat16
    x = np.ascontiguousarray(np.asarray(inputs["x"], dtype=np.float32))
    batch = np.asarray(inputs["batch"]).astype(np.int64)
    W1 = np.asarray(inputs["W1"], dtype=np.float32)
    b1 = np.asarray(inputs["b1"], dtype=np.float32)
    W2 = np.asarray(inputs["W2"], dtype=np.float32)
    b2 = np.asarray(inputs["b2"], dtype=np.float32)
    Wc = np.asarray(inputs["Wc"], dtype=np.float32).reshape(H, 1)
    bc = np.asarray(inputs["bc"], dtype=np.float32).reshape(1)

    n = batch.shape[0]
    counts, bounds, gsplit = _shard(batch)
    node_cnt = [int(bounds[gsplit[k + 1]] - bounds[gsplit[k]]) for k in range(N_CORES)]
    pad_unit = int(np.lcm.reduce([CHUNK * XB, OB, TB]))
    nodes_pad = int(-(-max(node_cnt) // pad_unit) * pad_unit)
    n_tb = nodes_pad // TB
    cpkc = 3 + G_PAD
    assert max(gsplit[k + 1] - gsplit[k] for k in range(N_CORES)) <= G_PAD

    key = nodes_pad
    if key not in _CACHE:
        _CACHE[key] = _build(nodes_pad)
    nc = _CACHE[key]

    iopk = np.ascontiguousarray(np.broadcast_to(
        np.arange(G_PAD, dtype=np.float32), (128, G_PAD)).astype(bf16))
    wpk = np.zeros((128, WPKC), dtype=bf16)
    wpk[:, W_W1:W_W1 + H] = W1.astype(bf16)
    wpk[:, W_W2A:W_W2A + H] = W2[0:128].astype(bf16)
    wpk[:, W_W2B:W_W2B + H] = W2[128:256].astype(bf16)
    wpk[:, W_WC] = Wc[0:128, 0].astype(bf16)
    wpk[:, W_WC + 1] = Wc[128:256, 0].astype(bf16)
    wpk[:, W_IO:W_IO + G_PAD] = np.broadcast_to(
        np.arange(G_PAD, dtype=np.float32), (128, G_PAD)).astype(bf16)
    wpk[0:G_PAD, W_B1:W_B1 + H] = b1.astype(bf16)
    wpk[0:G_PAD, W_B2:W_B2 + H] = b2.astype(bf16)

    n_groups = nodes_pad // (CHUNK * XB)
    in_maps = []
    for k in range(N_CORES):
        gs, ge = gsplit[k], gsplit[k + 1]
        ns, ne = int(bounds[gs]), int(bounds[ge])
        cnt = ne - ns
        ng = ge - gs
        bt = np.full(nodes_pad, G_PAD - 1, dtype=np.float32)
        bt[:cnt] = (batch[ns:ne] - gs).astype(np.float32)
        xbp = np.zeros((nodes_pad, ROW), dtype=bf16)
        xbp[:cnt, :D] = x[ns:ne].astype(bf16)
        xbp[:, D + 1] = bt.astype(bf16)  # high half of an f32 via bitcast
        # shuffle to (group, partition, chunk-in-group, row) DMA order
        xbp = np.ascontiguousarray(
            xbp.reshape(n_groups, XB, CHUNK, ROW).transpose(0, 2, 1, 3)
        ).reshape(nodes_pad, ROW)

        cpk = np.zeros((128, cpkc), dtype=np.float32)
        cpk[0:ng, C_NC] = counts[gs:ge].astype(np.float32)
        cpk[:, C_BC] = bc[0]
        cpk[0:G_PAD, 2] = np.arange(G_PAD, dtype=np.float32)
        cpk[0:G_PAD, 3:3 + G_PAD] = np.eye(G_PAD, dtype=np.float32)
        bt8 = np.full((1, nodes_pad), 255, dtype=np.uint8)
        bt8[0, :cnt] = (batch[ns:ne] - gs).astype(np.uint8)
        in_maps.append({"xb": xbp, "bt8": bt8, "iopk": iopk, "wpk": wpk,
                        "cpk": cpk})

    res = run_bass_kernel_spmd(nc, in_maps, core_ids=list(range(N_CORES)))
    outs = []
    for k in range(N_CORES):
        o = res.results[k]["out"].reshape(-1)
        outs.append(o[: node_cnt[k]])
    return np.concatenate(outs).reshape(n, 1).astype(np.float32)
